# revision 10
# baseline (speedup 1.0000x reference)
"""Multi-head attention + output Linear on 8 Trainium2 NeuronCores.

Problem: bs=2, seq=2048, embed=1024, heads=16, head_dim=64.
  out = Linear(softmax(mask(Q K^T / 8)) V)        (eval-mode dropout)

Sharding: core c in 0..7 handles batch b = c//4 and query block qb = c%4
(512 query rows), computing its exact [512, 1024] output slice - heads stay
together per core so the output Linear needs no cross-core reduction.

Per-core kernel v2 (Tile framework), all matmul I/O in bf16 (fp32 PSUM):
  scoresT[k, q] = K_h^T Q_h            (PE, d=64 contraction)
  probsT = exp(scoresT / 8)            (ACT, PSUM->SBUF bf16; the bottleneck)
  probsT *= maskT                      (DVE bf16, 2x mode)
  pv[q, 0:64] = probsT_chunk^T V_chunk (PE flipped: probs chunk stationary,
                                        V chunk moving: 64+1 rows per chunk
                                        instead of 512 -> PV cost / 8)
  pv[q, 64]  = probsT_chunk^T ones     (softmax denominator)
  attn = pv[:, 0:64] * recip(den)      (DVE recip + Pool per-partition scale)
  attnT chunks via PE transpose        (identity matmul, 128 rows/block)
  y += attnT^T W^T per head-pair       (PE, accumulated into SBUF by Pool adds
                                        so there is no serial tail after the
                                        exp stream ends)

Issue order software-pipelines PV/transpose/linear of head h-1 into the
ACT-bound exp stream of head h.
"""

import sys
import numpy as np

sys.path.insert(0, "/opt/trn_rl_repo")

import concourse.bass as bass
import concourse.tile as tile
from concourse import bacc, mybir
from concourse.bass_utils import run_bass_kernel_spmd

BS, SEQ, EMBED, HEADS = 2, 2048, 1024, 16
D = EMBED // HEADS            # 64
QB = SEQ // 4                 # 512 query rows per core
NC_COUNT = 8
KC = SEQ // 128               # 16 k chunks
F32 = mybir.dt.float32
BF16 = mybir.dt.bfloat16

_CACHE = {}

# exp-group sizes per head (chunks per ACT call); sum must be KC
GROUPS = [(c0, 2) for c0 in range(0, KC, 2)]


def _build_nc():
    nc = bacc.Bacc("TRN2", target_bir_lowering=False, debug=False)

    qT = nc.dram_tensor("qT", [HEADS, D, QB], BF16, kind="ExternalInput")
    kT = nc.dram_tensor("kT", [HEADS, D, SEQ], BF16, kind="ExternalInput")
    v = nc.dram_tensor("v", [SEQ, EMBED], BF16, kind="ExternalInput")
    m = nc.dram_tensor("m", [SEQ, QB], BF16, kind="ExternalInput")
    WT = nc.dram_tensor("WT", [EMBED, EMBED], BF16, kind="ExternalInput")
    bias = nc.dram_tensor("bias", [EMBED], F32, kind="ExternalInput")
    ident = nc.dram_tensor("ident", [128, 128], BF16, kind="ExternalInput")
    y = nc.dram_tensor("y", [QB, EMBED], F32, kind="ExternalOutput")

    m_re = m.rearrange("(c p) q -> p c q", p=128)
    v_re = v.rearrange("(c p) e -> p c e", p=128)

    with tile.TileContext(nc) as tc, \
         nc.allow_low_precision(reason="bf16 matmul inputs; fp32 accumulate in PSUM"):
        with tc.tile_pool(name="const", bufs=1) as const, \
             tc.tile_pool(name="kpool", bufs=2) as kpool, \
             tc.tile_pool(name="probs", bufs=2) as probs, \
             tc.tile_pool(name="asb", bufs=2) as asbp, \
             tc.tile_pool(name="vaug", bufs=2) as vaugp, \
             tc.tile_pool(name="rcp", bufs=4) as rcp, \
             tc.tile_pool(name="scps", bufs=2, space="PSUM") as scps, \
             tc.tile_pool(name="auxps", bufs=2, space="PSUM") as auxps, \
             tc.tile_pool(name="linps", bufs=2, space="PSUM") as linps:

            # ---- constants / big inputs (DMA order = need order) ----
            kTp = [None] * 8
            kTp[0] = kpool.tile([128, SEQ], BF16, tag="kT", name="kTp0")
            nc.sync.dma_start(out=kTp[0],
                              in_=kT[0:2].rearrange("h d s -> (h d) s"))
            qT_sb = const.tile([128, 8, QB], BF16)
            nc.sync.dma_start(
                out=qT_sb,
                in_=qT.rearrange("(hp two) d q -> (two d) hp q", two=2))
            mT_sb = const.tile([128, KC, QB], BF16)
            nc.sync.dma_start(out=mT_sb[:, 0:4], in_=m_re[:, 0:4])
            vfull = const.tile([128, KC, EMBED], BF16)
            nc.sync.dma_start(out=vfull[:, 0:4], in_=v_re[:, 0:4])
            ident_sb = const.tile([128, 128], BF16)
            nc.sync.dma_start(out=ident_sb, in_=ident[:, :])
            nc.sync.dma_start(out=vfull[:, 4:8], in_=v_re[:, 4:8])
            nc.sync.dma_start(out=mT_sb[:, 4:8], in_=m_re[:, 4:8])
            nc.sync.dma_start(out=vfull[:, 8:KC], in_=v_re[:, 8:KC])
            nc.sync.dma_start(out=mT_sb[:, 8:KC], in_=m_re[:, 8:KC])
            WT_sb = const.tile([128, 8, EMBED], BF16)
            nc.sync.dma_start(out=WT_sb,
                              in_=WT.rearrange("(c p) e -> p c e", p=128))
            # y accumulator, initialized with broadcast bias
            y_acc = const.tile([128, 4, EMBED], F32)
            bias_ap = bias[:]
            nc.sync.dma_start(
                out=y_acc,
                in_=bass.AP(tensor=bias_ap.tensor, offset=bias_ap.offset,
                            ap=[[0, 128], [0, 4]] + list(bias_ap.ap)))
            ones_sb = const.tile([128, KC], BF16)
            nc.vector.memset(ones_sb, 1.0)
            attnT = const.tile([128, 8, QB], BF16)

            probs_t = {}
            pv_t = {}
            asb_t = {}
            vaug_t = {}

            def issue_vaug(h):
                # [V_h | 1] moving tensor, assembled on Pool (SBUF-only op).
                # Slab-wise so early heads only wait on early v DMA slabs.
                t = vaugp.tile([128, KC, D + 1], BF16, tag="vaug",
                               name=f"vaug{h}")
                for s in range(0, KC, 4):
                    nc.gpsimd.tensor_copy(
                        t[:, s:s + 4, 0:D],
                        vfull[:, s:s + 4, h * D:(h + 1) * D])
                nc.gpsimd.tensor_copy(t[:, :, D], ones_sb)
                vaug_t[h] = t

            def issue_pv(k, part):
                pk = probs_t[k]
                if part == 0:
                    pv_t[k] = auxps.tile([128, 4, D + 1], F32, tag="pv",
                                         name=f"pv{k}")
                    # 8 accumulation groups share this bank: a start=True
                    # would zero the whole bank, so init once and accumulate
                    nc.vector.memset(pv_t[k], 0.0)
                c_range = range(0, 8) if part == 0 else range(8, KC)
                for c in c_range:
                    for qb in range(4):
                        nc.tensor.matmul(
                            pv_t[k][:, qb, :],
                            pk[:, c, qb * 128:(qb + 1) * 128],
                            vaug_t[k][:, c, :],
                            start=False, stop=(c == KC - 1),
                            skip_group_check=True)

            def issue_norm(k):
                # reciprocal of denom col; rescale into transpose staging
                hp_k, hh_k = k // 2, k % 2
                if hh_k == 0:
                    asb_t[hp_k] = asbp.tile([128, 4, 2, D], BF16, tag="asb", name=f"asb{hp_k}")
                rc = rcp.tile([128, 4], F32, tag="rc", name=f"rc{k}")
                nc.vector.reciprocal(rc, pv_t[k][:, :, D])
                for qb in range(4):
                    nc.vector.tensor_scalar_mul(
                        asb_t[hp_k][:, qb, hh_k, :],
                        pv_t[k][:, qb, 0:D],
                        rc[:, qb:qb + 1])
                del pv_t[k]
                del probs_t[k]
                del vaug_t[k]

            def issue_tr(hp_k):
                for qb in range(4):
                    trp = linps.tile([128, 128], BF16, tag="lin",
                                     name=f"tr{hp_k}_{qb}")
                    nc.tensor.transpose(trp, asb_t[hp_k][:, qb, :, :], ident_sb)
                    nc.vector.tensor_copy(
                        attnT[:, hp_k, qb * 128:(qb + 1) * 128], trp)
                del asb_t[hp_k]

            def issue_lin(qc, n, phase):
                lp = linps.tile([128, 512], F32, tag="lin",
                                name=f"lin{phase}_{qc}_{n}")
                for i, hpi in enumerate(range(4 * phase, 4 * phase + 4)):
                    nc.tensor.matmul(
                        lp,
                        attnT[:, hpi, qc * 128:(qc + 1) * 128],
                        WT_sb[:, hpi, n * 512:(n + 1) * 512],
                        start=(i == 0), stop=(i == 3))
                nc.vector.tensor_add(
                    y_acc[:, qc, n * 512:(n + 1) * 512], lp,
                    y_acc[:, qc, n * 512:(n + 1) * 512])

            pending_pv = None
            pending_tr = None
            issue_vaug(0)

            for h in range(HEADS):
                hp, hh = h // 2, h % 2
                if h + 1 < HEADS:
                    issue_vaug(h + 1)
                if hh == 0 and hp + 1 < 8:
                    kTp[hp + 1] = kpool.tile([128, SEQ], BF16, tag="kT",
                                              name=f"kTp{hp + 1}")
                    nc.sync.dma_start(
                        out=kTp[hp + 1],
                        in_=kT[2 * hp + 2:2 * hp + 4].rearrange(
                            "h d s -> (h d) s"))
                probs_t[h] = probs.tile([128, KC, QB], BF16, tag="probs", name=f"probs{h}")
                for gi, (c0, gsz) in enumerate(GROUPS):
                    sc = scps.tile([128, gsz, QB], F32, tag="sc")
                    for j in range(gsz):
                        c = c0 + j
                        nc.tensor.matmul(
                            sc[:, j, :],
                            kTp[hp][hh * D:(hh + 1) * D, c * 128:(c + 1) * 128],
                            qT_sb[hh * D:(hh + 1) * D, hp, :],
                            start=True, stop=True)
                    nc.scalar.activation(
                        out=probs_t[h][:, c0:c0 + gsz, :], in_=sc,
                        func=mybir.ActivationFunctionType.Exp,
                        scale=float(1.0 / np.sqrt(D)))
                    meng = nc.gpsimd if (h * len(GROUPS) + gi) % 6 == 5 \
                        else nc.vector
                    meng.tensor_mul(
                        probs_t[h][:, c0:c0 + gsz, :],
                        probs_t[h][:, c0:c0 + gsz, :],
                        mT_sb[:, c0:c0 + gsz, :])
                    if gi == 1 and pending_pv is not None:
                        issue_pv(pending_pv, 0)
                    elif gi == 3 and pending_pv is not None:
                        issue_pv(pending_pv, 1)
                    elif gi == 4 and pending_pv is not None:
                        issue_norm(pending_pv)
                    elif gi == 5 and pending_tr is not None:
                        issue_tr(pending_tr)
                        pending_tr = None
                    elif gi == 6 and h in (9, 10, 11, 12):
                        for n in range(2):
                            issue_lin(h - 9, n, 0)
                if pending_pv is not None and pending_pv % 2 == 1:
                    pending_tr = pending_pv // 2
                pending_pv = h

            # flush: PV + norm for head 15, transpose + linear phase B
            issue_pv(15, 0)
            issue_pv(15, 1)
            issue_norm(15)
            issue_tr(7)
            for qc in range(4):
                for n in range(2):
                    issue_lin(qc, n, 1)
            for qc in range(4):
                nc.sync.dma_start(out=y[qc * 128:(qc + 1) * 128, :],
                                  in_=y_acc[:, qc, :])

    nc.compile()
    return nc


def _prep_in_maps(q, k, v, padding_mask, W, b):
    import ml_dtypes
    bf = ml_dtypes.bfloat16
    q = np.asarray(q, dtype=np.float32)
    k = np.asarray(k, dtype=np.float32)
    v = np.asarray(v, dtype=np.float32)
    m = np.asarray(padding_mask)
    W = np.asarray(W, dtype=np.float32)
    b = np.asarray(b, dtype=np.float32)

    # [bs, seq, embed] -> [bs, heads, d, seq]
    qT = np.ascontiguousarray(
        q.reshape(BS, SEQ, HEADS, D).transpose(0, 2, 3, 1).astype(bf))
    kT = np.ascontiguousarray(
        k.reshape(BS, SEQ, HEADS, D).transpose(0, 2, 3, 1).astype(bf))
    vb = np.ascontiguousarray(v.astype(bf))
    # mask [bs, 1, q, k] -> [bs, k, q] in bf16 (0/1)
    mT = np.ascontiguousarray(m[:, 0].transpose(0, 2, 1).astype(bf))
    WTc = np.ascontiguousarray(W.T.astype(bf))
    ident = np.eye(128, dtype=bf)

    in_maps = []
    for c in range(NC_COUNT):
        bi, qb = c // 4, c % 4
        in_maps.append({
            "qT": np.ascontiguousarray(qT[bi, :, :, qb * QB:(qb + 1) * QB]),
            "kT": kT[bi],
            "v": vb[bi],
            "m": np.ascontiguousarray(mT[bi, :, qb * QB:(qb + 1) * QB]),
            "WT": WTc,
            "bias": b,
            "ident": ident,
        })
    return in_maps


def _run(in_maps, **kw):
    if "nc" not in _CACHE:
        _CACHE["nc"] = _build_nc()
    return run_bass_kernel_spmd(_CACHE["nc"], in_maps, list(range(NC_COUNT)), **kw)


def kernel(q, k, v, padding_mask, W, b):
    in_maps = _prep_in_maps(q, k, v, padding_mask, W, b)
    res = _run(in_maps)
    out = np.empty((BS, SEQ, EMBED), dtype=np.float32)
    for c in range(NC_COUNT):
        bi, qb = c // 4, c % 4
        out[bi, qb * QB:(qb + 1) * QB] = res.results[c]["y"]
    return out


# revision 12
# speedup vs baseline: 1.0059x; 1.0059x over previous
"""Multi-head attention + output Linear on 8 Trainium2 NeuronCores.

Problem: bs=2, seq=2048, embed=1024, heads=16, head_dim=64.
  out = Linear(softmax(mask(Q K^T / 8)) V)        (eval-mode dropout)

Sharding: core c in 0..7 handles batch b = c//4 and query block qb = c%4
(512 query rows), computing its exact [512, 1024] output slice - heads stay
together per core so the output Linear needs no cross-core reduction.

Per-core kernel v2 (Tile framework), all matmul I/O in bf16 (fp32 PSUM):
  scoresT[k, q] = K_h^T Q_h            (PE, d=64 contraction)
  probsT = exp(scoresT / 8)            (ACT, PSUM->SBUF bf16; the bottleneck)
  probsT *= maskT                      (DVE bf16, 2x mode)
  pv[q, 0:64] = probsT_chunk^T V_chunk (PE flipped: probs chunk stationary,
                                        V chunk moving: 64+1 rows per chunk
                                        instead of 512 -> PV cost / 8)
  pv[q, 64]  = probsT_chunk^T ones     (softmax denominator)
  attn = pv[:, 0:64] * recip(den)      (DVE recip + Pool per-partition scale)
  attnT chunks via PE transpose        (identity matmul, 128 rows/block)
  y += attnT^T W^T per head-pair       (PE, accumulated into SBUF by Pool adds
                                        so there is no serial tail after the
                                        exp stream ends)

Issue order software-pipelines PV/transpose/linear of head h-1 into the
ACT-bound exp stream of head h.
"""

import sys
import numpy as np

sys.path.insert(0, "/opt/trn_rl_repo")

import concourse.bass as bass
import concourse.tile as tile
from concourse import bacc, mybir
from concourse.bass_utils import run_bass_kernel_spmd

BS, SEQ, EMBED, HEADS = 2, 2048, 1024, 16
D = EMBED // HEADS            # 64
QB = SEQ // 4                 # 512 query rows per core
NC_COUNT = 8
KC = SEQ // 128               # 16 k chunks
F32 = mybir.dt.float32
BF16 = mybir.dt.bfloat16

_CACHE = {}

# exp-group sizes per head (chunks per ACT call); sum must be KC
GROUPS = [(c0, 2) for c0 in range(0, KC, 2)]


def _build_nc():
    nc = bacc.Bacc("TRN2", target_bir_lowering=False, debug=False)

    qT = nc.dram_tensor("qT", [HEADS, D, QB], BF16, kind="ExternalInput")
    kT = nc.dram_tensor("kT", [HEADS, D, SEQ], BF16, kind="ExternalInput")
    v = nc.dram_tensor("v", [SEQ, EMBED], BF16, kind="ExternalInput")
    m = nc.dram_tensor("m", [SEQ, QB], BF16, kind="ExternalInput")
    WT = nc.dram_tensor("WT", [EMBED, EMBED], BF16, kind="ExternalInput")
    bias = nc.dram_tensor("bias", [EMBED], F32, kind="ExternalInput")
    ident = nc.dram_tensor("ident", [128, 128], BF16, kind="ExternalInput")
    y = nc.dram_tensor("y", [QB, EMBED], F32, kind="ExternalOutput")

    m_re = m.rearrange("(c p) q -> p c q", p=128)
    v_re = v.rearrange("(c p) e -> p c e", p=128)

    with tile.TileContext(nc) as tc, \
         nc.allow_low_precision(reason="bf16 matmul inputs; fp32 accumulate in PSUM"):
        with tc.tile_pool(name="const", bufs=1) as const, \
             tc.tile_pool(name="kpool", bufs=2) as kpool, \
             tc.tile_pool(name="probs", bufs=2) as probs, \
             tc.tile_pool(name="asb", bufs=2) as asbp, \
             tc.tile_pool(name="vaug", bufs=3) as vaugp, \
             tc.tile_pool(name="rcp", bufs=4) as rcp, \
             tc.tile_pool(name="scps", bufs=2, space="PSUM") as scps, \
             tc.tile_pool(name="auxps", bufs=2, space="PSUM") as auxps, \
             tc.tile_pool(name="linps", bufs=2, space="PSUM") as linps:

            # ---- constants / big inputs (DMA order = need order) ----
            kTp = [None] * 8
            kTp[0] = kpool.tile([128, SEQ], BF16, tag="kT", name="kTp0")
            nc.sync.dma_start(out=kTp[0],
                              in_=kT[0:2].rearrange("h d s -> (h d) s"))
            qT_sb = const.tile([128, 8, QB], BF16)
            nc.sync.dma_start(
                out=qT_sb,
                in_=qT.rearrange("(hp two) d q -> (two d) hp q", two=2))
            mT_sb = const.tile([128, KC, QB], BF16)
            nc.sync.dma_start(out=mT_sb[:, 0:4], in_=m_re[:, 0:4])
            vfull = const.tile([128, KC, EMBED], BF16)
            nc.sync.dma_start(out=vfull[:, 0:4], in_=v_re[:, 0:4])
            ident_sb = const.tile([128, 128], BF16)
            nc.sync.dma_start(out=ident_sb, in_=ident[:, :])
            nc.sync.dma_start(out=vfull[:, 4:8], in_=v_re[:, 4:8])
            nc.sync.dma_start(out=mT_sb[:, 4:8], in_=m_re[:, 4:8])
            nc.sync.dma_start(out=vfull[:, 8:KC], in_=v_re[:, 8:KC])
            nc.sync.dma_start(out=mT_sb[:, 8:KC], in_=m_re[:, 8:KC])
            WT_sb = const.tile([128, 8, EMBED], BF16)
            nc.sync.dma_start(out=WT_sb,
                              in_=WT.rearrange("(c p) e -> p c e", p=128))
            # y accumulator, initialized with broadcast bias
            y_acc = const.tile([128, 4, EMBED], F32)
            bias_ap = bias[:]
            nc.sync.dma_start(
                out=y_acc,
                in_=bass.AP(tensor=bias_ap.tensor, offset=bias_ap.offset,
                            ap=[[0, 128], [0, 4]] + list(bias_ap.ap)))
            ones_sb = const.tile([128, KC], BF16)
            nc.vector.memset(ones_sb, 1.0)
            attnT = const.tile([128, 8, QB], BF16)

            probs_t = {}
            pv_t = {}
            asb_t = {}
            vaug_t = {}

            def issue_vaug(h):
                # [V_h | 1] moving tensor, assembled on Pool (SBUF-only op).
                # Slab-wise so early heads only wait on early v DMA slabs.
                t = vaugp.tile([128, KC, D + 1], BF16, tag="vaug",
                               name=f"vaug{h}")
                for s in range(0, KC, 4):
                    nc.gpsimd.tensor_copy(
                        t[:, s:s + 4, 0:D],
                        vfull[:, s:s + 4, h * D:(h + 1) * D])
                nc.gpsimd.tensor_copy(t[:, :, D], ones_sb)
                vaug_t[h] = t

            def issue_pv(k, part):
                pk = probs_t[k]
                if part == 0:
                    pv_t[k] = auxps.tile([128, 4, D + 1], F32, tag="pv",
                                         name=f"pv{k}")
                    # 8 accumulation groups share this bank: a start=True
                    # would zero the whole bank, so init once and accumulate
                    nc.vector.memset(pv_t[k], 0.0)
                c_range = range(0, 8) if part == 0 else range(8, KC)
                for c in c_range:
                    for qb in range(4):
                        nc.tensor.matmul(
                            pv_t[k][:, qb, :],
                            pk[:, c, qb * 128:(qb + 1) * 128],
                            vaug_t[k][:, c, :],
                            start=False, stop=(c == KC - 1),
                            skip_group_check=True)

            def issue_norm(k):
                # reciprocal of denom col; rescale into transpose staging
                hp_k, hh_k = k // 2, k % 2
                if hh_k == 0:
                    asb_t[hp_k] = asbp.tile([128, 4, 2, D], BF16, tag="asb", name=f"asb{hp_k}")
                rc = rcp.tile([128, 4], F32, tag="rc", name=f"rc{k}")
                nc.vector.reciprocal(rc, pv_t[k][:, :, D])
                for qb in range(4):
                    nc.vector.tensor_scalar_mul(
                        asb_t[hp_k][:, qb, hh_k, :],
                        pv_t[k][:, qb, 0:D],
                        rc[:, qb:qb + 1])
                del pv_t[k]
                del probs_t[k]
                del vaug_t[k]

            def issue_tr(hp_k):
                for qb in range(4):
                    trp = linps.tile([128, 128], BF16, tag="lin",
                                     name=f"tr{hp_k}_{qb}")
                    nc.tensor.transpose(trp, asb_t[hp_k][:, qb, :, :], ident_sb)
                    nc.vector.tensor_copy(
                        attnT[:, hp_k, qb * 128:(qb + 1) * 128], trp)
                del asb_t[hp_k]

            def issue_lin(qc, n, phase):
                lp = linps.tile([128, 512], F32, tag="lin",
                                name=f"lin{phase}_{qc}_{n}")
                for i, hpi in enumerate(range(4 * phase, 4 * phase + 4)):
                    nc.tensor.matmul(
                        lp,
                        attnT[:, hpi, qc * 128:(qc + 1) * 128],
                        WT_sb[:, hpi, n * 512:(n + 1) * 512],
                        start=(i == 0), stop=(i == 3))
                nc.vector.tensor_add(
                    y_acc[:, qc, n * 512:(n + 1) * 512], lp,
                    y_acc[:, qc, n * 512:(n + 1) * 512])

            pending_pv = None
            pending_tr = None
            issue_vaug(0)

            for h in range(HEADS):
                hp, hh = h // 2, h % 2
                if hh == 0 and hp + 1 < 8:
                    kTp[hp + 1] = kpool.tile([128, SEQ], BF16, tag="kT",
                                              name=f"kTp{hp + 1}")
                    nc.sync.dma_start(
                        out=kTp[hp + 1],
                        in_=kT[2 * hp + 2:2 * hp + 4].rearrange(
                            "h d s -> (h d) s"))
                probs_t[h] = probs.tile([128, KC, QB], BF16, tag="probs", name=f"probs{h}")
                for gi, (c0, gsz) in enumerate(GROUPS):
                    sc = scps.tile([128, gsz, QB], F32, tag="sc")
                    for j in range(gsz):
                        c = c0 + j
                        nc.tensor.matmul(
                            sc[:, j, :],
                            kTp[hp][hh * D:(hh + 1) * D, c * 128:(c + 1) * 128],
                            qT_sb[hh * D:(hh + 1) * D, hp, :],
                            start=True, stop=True)
                    nc.scalar.activation(
                        out=probs_t[h][:, c0:c0 + gsz, :], in_=sc,
                        func=mybir.ActivationFunctionType.Exp,
                        scale=float(1.0 / np.sqrt(D)))
                    meng = nc.gpsimd if (h * len(GROUPS) + gi) % 6 == 5 \
                        else nc.vector
                    meng.tensor_mul(
                        probs_t[h][:, c0:c0 + gsz, :],
                        probs_t[h][:, c0:c0 + gsz, :],
                        mT_sb[:, c0:c0 + gsz, :])
                    if gi == 1 and pending_pv is not None:
                        issue_pv(pending_pv, 0)
                    elif gi == 3 and pending_pv is not None:
                        issue_pv(pending_pv, 1)
                    elif gi == 4:
                        if pending_pv is not None:
                            issue_norm(pending_pv)
                        if h + 1 < HEADS:
                            issue_vaug(h + 1)
                    elif gi == 5 and pending_tr is not None:
                        issue_tr(pending_tr)
                        pending_tr = None
                    elif gi == 6 and h in (9, 10, 11, 12):
                        for n in range(2):
                            issue_lin(h - 9, n, 0)
                if pending_pv is not None and pending_pv % 2 == 1:
                    pending_tr = pending_pv // 2
                pending_pv = h

            # flush: PV + norm for head 15, transpose + linear phase B
            issue_pv(15, 0)
            issue_pv(15, 1)
            issue_norm(15)
            issue_tr(7)
            for qc in range(4):
                for n in range(2):
                    issue_lin(qc, n, 1)
            for qc in range(4):
                nc.sync.dma_start(out=y[qc * 128:(qc + 1) * 128, :],
                                  in_=y_acc[:, qc, :])

    nc.compile()
    return nc


def _prep_in_maps(q, k, v, padding_mask, W, b):
    import ml_dtypes
    bf = ml_dtypes.bfloat16
    q = np.asarray(q, dtype=np.float32)
    k = np.asarray(k, dtype=np.float32)
    v = np.asarray(v, dtype=np.float32)
    m = np.asarray(padding_mask)
    W = np.asarray(W, dtype=np.float32)
    b = np.asarray(b, dtype=np.float32)

    # [bs, seq, embed] -> [bs, heads, d, seq]
    qT = np.ascontiguousarray(
        q.reshape(BS, SEQ, HEADS, D).transpose(0, 2, 3, 1).astype(bf))
    kT = np.ascontiguousarray(
        k.reshape(BS, SEQ, HEADS, D).transpose(0, 2, 3, 1).astype(bf))
    vb = np.ascontiguousarray(v.astype(bf))
    # mask [bs, 1, q, k] -> [bs, k, q] in bf16 (0/1)
    mT = np.ascontiguousarray(m[:, 0].transpose(0, 2, 1).astype(bf))
    WTc = np.ascontiguousarray(W.T.astype(bf))
    ident = np.eye(128, dtype=bf)

    in_maps = []
    for c in range(NC_COUNT):
        bi, qb = c // 4, c % 4
        in_maps.append({
            "qT": np.ascontiguousarray(qT[bi, :, :, qb * QB:(qb + 1) * QB]),
            "kT": kT[bi],
            "v": vb[bi],
            "m": np.ascontiguousarray(mT[bi, :, qb * QB:(qb + 1) * QB]),
            "WT": WTc,
            "bias": b,
            "ident": ident,
        })
    return in_maps


def _run(in_maps, **kw):
    if "nc" not in _CACHE:
        _CACHE["nc"] = _build_nc()
    return run_bass_kernel_spmd(_CACHE["nc"], in_maps, list(range(NC_COUNT)), **kw)


def kernel(q, k, v, padding_mask, W, b):
    in_maps = _prep_in_maps(q, k, v, padding_mask, W, b)
    res = _run(in_maps)
    out = np.empty((BS, SEQ, EMBED), dtype=np.float32)
    for c in range(NC_COUNT):
        bi, qb = c // 4, c % 4
        out[bi, qb * QB:(qb + 1) * QB] = res.results[c]["y"]
    return out
